# revision 68
# baseline (speedup 1.0000x reference)
"""Trainium2 Bass kernel: single-head causal attention.

  out[b] = softmax(mask((q[b]Wq+bq)(k[b]Wk+bk)^T / sqrt(dk))) (v[b]Wv+bv)

Sharding: data-parallel over batch, one batch element per NeuronCore (B=8,
n_cores=8). No collectives. Host-side prep is layout + dtype staging +
parameter re-layout / algebraic folding:
  - q, k are pre-cast to fp8 e4m3 on host (2MB each per core vs 8MB f32);
    v stays bf16 (4MB) -- fp8 v alone pushes rel err past the 2e-2 gate.
  - Wq*scale and the bq-augmented Wk are fp8 with a x64 gain (their natural
    range is below the e4m3 normal minimum); the gain cancels inside exp()
    via scale=2^-12, with the qpT ones-row at 64 to keep the folded-bq
    column consistent. Wk is zero-padded to 128 columns (DoubleRow needs a
    32/64/128 stationary width; matmul time only depends on moving size).
  - bk drops out (softmax-invariant); bv is added after normalization.
  - End-to-end rel err 1.52e-2 (gate 2e-2), bit-matched by numsim3.py.

Per-core dataflow (S=2048, D=1024, dk=64, P=128):
  - inputs stream on the single-FIFO SWDGE ring in order k0 (2 half-DMAs:
    completion latency is per-DMA, so proj starts mid-arrival), q0 (2),
    kq1, kq2, v0, kq3, v1..v3. v0 is raised ahead of kq3: the PE only
    reaches the c3 projections ~3us after kq3 lands anyway, and early v0
    gives the PE vp/out work to fill exp-drain stalls.
  - k/q projections are fp8 DoubleRow (2 d-tiles = 256 contraction rows
    per matmul): half the PE cycles of bf16. 2-deep proj psum pool so the
    DVE psum-copy never stalls the next projection.
  - scores are emitted as PAIRS of k-tiles into [P, 2*CHUNK] 2-bank psum
    tiles; ONE ACTIVATE exps both pieces (3D AP into the single u_big
    [P, 16, S] tile) -- the ACT exp stream is the mid-kernel rate limiter
    and per-instruction overhead (~250ns) matters. The <=128 garbage
    below-diagonal columns a pair adds are never read downstream. Causal
    diagonal blocks masked by a 0/1 upper-tri multiply on DVE.
  - The kernel is PE-cycle-bound at the HAM-throttled ~1.2GHz clock
    (short 2.4GHz grants arrive sporadically), so emission order finely
    interleaves score pairs (exp-lockstepped) with vp projections and
    output tiles by expected data arrival, keeping PE and ACT both fed.
    A 6-matmul warm-up on a memset tile bridges preamble -> first data
    and opens the PE p-state ramp.
  - out[sq-tile j] = sum_t u[t]^T @ vp_t in PSUM [128,65]; vp column 64 is
    1 so the output matmul also emits the softmax denominator. j=14,15
    partial sums (t<=13) are accumulated one v-chunk early so only ~1us of
    compute trails the final DMA byte. Normalize with vector.reciprocal +
    tensor_scalar_mul, += bv, store on the sync HWDGE ring (off the ACT
    queue, which exp saturates).
  - tile_wait_until group tags keep Tile's static schedule aligned with
    the real FIFO arrival order (pool slots are granted in emission order,
    so emission must also match).
"""

import sys
from contextlib import ExitStack

import numpy as np

sys.path.insert(0, "/opt/trn_rl_repo")

import ml_dtypes  # noqa: E402

import concourse.mybir as mybir  # noqa: E402
import concourse.tile as tile  # noqa: E402
from concourse import bacc  # noqa: E402
from concourse.bass import ds, ts  # noqa: E402
from concourse.bass_utils import run_bass_kernel_spmd  # noqa: E402

S = 2048
D = 1024
DK = 64
P = 128
NDT = D // P  # 8 d-model tiles
NST = S // P  # 16 seq tiles
CHUNK = 512  # seq chunk = matmul moving-operand / PSUM-bank free size
NCH = S // CHUNK  # 4 column chunks for k/q
B = 8
NCORES = 8

# v chunk column spans (start, len): last chunk kept small so little work
# trails the final DMA byte
VCH = [(0, 512), (512, 512), (1024, 768), (1792, 256)]

F32 = mybir.dt.float32
BF16 = mybir.dt.bfloat16
F8 = mybir.dt.float8e4
BF = ml_dtypes.bfloat16
F8NP = ml_dtypes.float8_e4m3

# schedule group ticks (tile_wait_until "ms" units, used as logical order).
# k0 and q0 are separate loads (early exp start); k_c/q_c for c>=1 are
# merged interleaved pair-loads (fewer per-DMA completion stalls). v0 is
# streamed BEFORE kq3: the PE's queue only reaches the c3 projections ~3us
# after kq3 would land anyway, and an early v0 gives the PE vp/out work to
# fill the exp-drain stalls of the scores phase.
G_K = [0.01, 0.03, 0.04, 0.055]
G_Q = [0.02, 0.03, 0.04, 0.055]
G_V = [0.045, 0.07, 0.08, 0.09]


def build(variant: str = "causal") -> bacc.Bacc:
    """variant: 'causal' (skip sk>sq tiles, tri-mask diagonal),
    'full' (no masking), 'general' (arbitrary multiplicative mask)."""
    assert variant in ("causal", "full", "general")
    causal = variant == "causal"

    nc = bacc.Bacc()
    # k0/q0 are stored half-major ([P, 2, NDT, 256]) and loaded as two DMAs
    # each: SWDGE completion latency is per-DMA, so the first projection can
    # start ~halfway through the chunk's arrival.
    HC = CHUNK // 2
    k0_d = nc.declare_dram_parameter("k0", [P, 2, NDT, HC], F8, isOutput=False)
    q0_d = nc.declare_dram_parameter("q0", [P, 2, NDT, HC], F8, isOutput=False)
    kq_ds = [
        nc.declare_dram_parameter(f"kq{c}", [P, 2, NDT, CHUNK], F8, isOutput=False)
        for c in range(1, NCH)
    ]
    vT_ds = [
        nc.declare_dram_parameter(f"vT{i}", [P, NDT, L], BF16, isOutput=False)
        for i, (_, L) in enumerate(VCH)
    ]
    wq_d = nc.declare_dram_parameter("wq", [P, NDT * DK], F8, isOutput=False)
    # wk is padded to 128 columns (64 real + bq-fold + 63 zeros): DoubleRow
    # requires the stationary free size to be 32/64/128, and matmul time only
    # depends on the moving size, so the padding is free.
    wk_d = nc.declare_dram_parameter("wk", [P, NDT * P], F8, isOutput=False)
    wv_d = nc.declare_dram_parameter("wv", [P, NDT * DK], BF16, isOutput=False)
    bvb_d = nc.declare_dram_parameter("bvb", [P, DK], F32, isOutput=False)
    if causal:
        m01_d = nc.declare_dram_parameter("m01", [P, P], BF16, isOutput=False)
    if variant == "general":
        mT_d = nc.declare_dram_parameter("mT", [S, S], BF16, isOutput=False)
    out_d = nc.declare_dram_parameter("out", [S, DK], F32, isOutput=True)

    with ExitStack() as ctx:
        tc = ctx.enter_context(tile.TileContext(nc))
        const_pool = ctx.enter_context(tc.tile_pool(name="const", bufs=1))
        ld_pool = ctx.enter_context(tc.tile_pool(name="loads", bufs=1))
        pp_pool = ctx.enter_context(tc.tile_pool(name="projT", bufs=1))
        u_pool = ctx.enter_context(tc.tile_pool(name="u", bufs=1))
        vp_pool = ctx.enter_context(tc.tile_pool(name="vp", bufs=1))
        osb_pool = ctx.enter_context(tc.tile_pool(name="osb", bufs=1))
        scr_pool = ctx.enter_context(tc.tile_pool(name="scr", bufs=1))
        # proj pool has 2 bufs so consecutive projections (k/q alternation,
        # c0 halves) overlap their DVE psum-copy instead of stalling the PE
        # ~780ns each; the vp projections reuse this pool (projs are done by
        # the time v arrives).
        ps_proj = ctx.enter_context(tc.tile_pool(name="ps_proj", bufs=2, space="PSUM"))
        # scores psum: [P, 2*CHUNK] double tiles (2 banks) so one ACTIVATE
        # exps two score pieces — the ACT exp stream is the mid-kernel rate
        # limiter and its per-instruction overhead (~250ns) is substantial.
        ps_sc = ctx.enter_context(tc.tile_pool(name="ps_sc", bufs=2, space="PSUM"))
        ps_out = ctx.enter_context(tc.tile_pool(name="ps_out", bufs=2, space="PSUM"))

        # Warm-tile memset is the FIRST DVE instruction so the PE warm-up has
        # no dependency on any DMA (the observation reads of the consts would
        # otherwise block the DVE queue until the const ring delivers).
        warm = const_pool.tile([P, CHUNK], BF16, name="warm")
        nc.vector.memset(warm[:, :], 0.125)

        # --- constants (HWDGE sync ring), ordered by first use:
        # wk (proj k0 ~12us) -> wq (proj q0) -> m01 (first diag mask) ->
        # wv (vp, after the kq stream) -> bvb (norm, later still).
        wk_sb = const_pool.tile([P, NDT * P], F8, name="wk_sb")
        nc.sync.dma_start(wk_sb[:, :], wk_d[:, :])
        wq_sb = const_pool.tile([P, NDT * DK], F8, name="wq_sb")
        nc.sync.dma_start(wq_sb[:, :], wq_d[:, :])
        if causal:
            m01_sb = const_pool.tile([P, P], BF16, name="m01_sb")
            nc.sync.dma_start(m01_sb[:, :], m01_d[:, :])
        wv_sb = const_pool.tile([P, NDT * DK], BF16, name="wv_sb")
        nc.sync.dma_start(wv_sb[:, :], wv_d[:, :])
        bvb_sb = const_pool.tile([P, DK], F32, name="bvb_sb")
        nc.sync.dma_start(bvb_sb[:, :], bvb_d[:, :])

        scr = scr_pool.tile([P, 4], F32, name="scr")

        # --- big input loads ------------------------------------------------
        # SWDGE single FIFO ring with f32->bf16 cast in flight, strict order
        # k0, q0, kq1, kq2, kq3, v0..v3. k and q share one SBUF tile so the
        # merged pair chunks land in a single DMA.
        kqt = ld_pool.tile([P, 2 * NDT * S], F8, tag="kqt", name="kqt")
        vt = ld_pool.tile([P, NDT * S], BF16, tag="vt", name="vt")
        kq4 = kqt[:, :].rearrange("p (w t s) -> p w t s", w=2, s=S)
        kt3 = kq4[:, 0]
        qt3 = kq4[:, 1]
        vt3 = vt[:, :].rearrange("p (t s) -> p t s", s=S)

        with tc.tile_wait_until(G_K[0]):
            nc.gpsimd.dma_start(kt3[:, :, ds(0, HC)], k0_d[:, 0])
            nc.gpsimd.dma_start(kt3[:, :, ds(HC, HC)], k0_d[:, 1])
        with tc.tile_wait_until(G_Q[0]):
            nc.gpsimd.dma_start(qt3[:, :, ds(0, HC)], q0_d[:, 0])
            nc.gpsimd.dma_start(qt3[:, :, ds(HC, HC)], q0_d[:, 1])
        def load_kq(c):
            with tc.tile_wait_until(G_K[c]):
                nc.gpsimd.dma_start(
                    kq4[:, :, :, ds(c * CHUNK, CHUNK)], kq_ds[c - 1][:, :, :, :]
                )

        def load_v(i):
            a, L = VCH[i]
            with tc.tile_wait_until(G_V[i]):
                nc.gpsimd.dma_start(vt3[:, :, ds(a, L)], vT_ds[i][:, :, :])

        # ring FIFO order = emission order: kq1, kq2, v0, kq3, v1..v3
        load_kq(1)
        load_kq(2)
        load_v(0)
        load_kq(3)
        for i in range(1, len(VCH)):
            load_v(i)

        # PE warm-up: throwaway matmuls on a memset tile (no DMA dependency,
        # so they start right after the NEFF entry barrier) to open the PE
        # p-state ramp (0.65 -> 1.2 -> 2.4 GHz after ~3us of continuous
        # execution) before real work arrives, and to bridge the gap until
        # the first k chunk lands.
        with tc.tile_wait_until(0.005):
            wps = None
            for _ in range(6):
                wps = ps_sc.tile([P, 2 * CHUNK], F32, tag="ps_sc", name="ps_warm")
                nc.tensor.matmul(
                    wps[:, ds(0, CHUNK)],
                    lhsT=warm[:, ds(0, P)],
                    rhs=warm[:, ds(0, CHUNK)],
                    start=True,
                    stop=True,
                )
            nc.vector.tensor_copy(scr[:, ds(2, 1)], wps[:, ds(0, 1)])

        qpT = pp_pool.tile([DK + 1, S], BF16, tag="qpT", name="qpT")
        kpT = pp_pool.tile([DK + 1, S], BF16, tag="kpT", name="kpT")
        # weights carry a x64 gain (fp8 subnormal dodge); the ones-row of qpT
        # must match so the bq column lands at the same 64*64 scale as the
        # qp.kp products. exp() de-scales by 2^-12.
        nc.vector.memset(qpT[ds(DK, 1), :], 64.0)

        wq3 = wq_sb[:, :].rearrange("p (t m) -> p t m", m=DK)
        wk3 = wk_sb[:, :].rearrange("p (t m) -> p t m", m=P)

        def proj_chunk(src3, w3, dst, m, c, mm, halves=1):
            # fp8 DoubleRow: two d-tiles (256 contraction rows) per matmul.
            # mm = stationary free width (64 for q, 128 for padded k); m = rows
            # actually copied out. halves=2 projects the chunk in two column
            # halves (for c0, whose data lands as two DMAs).
            ps = ps_proj.tile([P, CHUNK], F32, tag="ps_proj", name="ps_p")
            L = CHUNK // halves
            for h in range(halves):
                for d in range(0, NDT, 2):
                    nc.tensor.matmul(
                        ps[:mm, ds(h * L, L)],
                        lhsT=w3[:, ds(d, 2), :],
                        rhs=src3[:, ds(d, 2), ds(c * CHUNK + h * L, L)],
                        start=(d == 0),
                        stop=(d == NDT - 2),
                        perf_mode=mybir.MatmulPerfMode.DoubleRow,
                    )
                nc.vector.tensor_copy(
                    dst[:m, ds(c * CHUNK + h * L, L)], ps[:m, ds(h * L, L)]
                )

        # --- scores + exp, sq-chunk-major so exp trails the q chunks -------
        if variant == "general":
            # mask slices are streamed per score pair (a resident [S, S] mask
            # no longer fits SBUF next to u_big); slow but correct.
            mt_pool = ctx.enter_context(tc.tile_pool(name="mt", bufs=2))

        # u is one big [P, NST, S] tile (columns = absolute sq) so a single
        # 3D-AP ACTIVATE can exp a PAIR of score pieces at once.
        u_big = u_pool.tile([P, NST * S], BF16, tag="u_big", name="u_big")
        u3 = u_big[:, :].rearrange("p (t s) -> p t s", s=S)

        def sc_pair(cq, i):
            """score matmuls + one shared exp for the piece pair
            (q-chunk cq, k-tiles 2i and 2i+1). Both pieces are computed from
            the same column start a (the earlier tile's causal start); the up
            to 128 below-diagonal columns this adds to tile 2i+1 are garbage
            that no out matmul ever reads."""
            t0 = 2 * i
            a = max(cq * CHUNK, t0 * P) if causal else cq * CHUNK
            w = (cq + 1) * CHUNK - a
            ps = ps_sc.tile([P, 2 * CHUNK], F32, tag="ps_sc", name="ps_s")
            for h in (0, 1):
                nc.tensor.matmul(
                    ps[:, ds(h * CHUNK, w)],
                    lhsT=kpT[:, ds((t0 + h) * P, P)],
                    rhs=qpT[:, ds(a, w)],
                    start=True,
                    stop=True,
                )
            ps3 = ps[:, :].rearrange("p (n s) -> p n s", n=2)
            nc.scalar.activation(
                u3[:, ds(t0, 2), ds(a, w)],
                ps3[:, :, ds(0, w)],
                mybir.ActivationFunctionType.Exp,
                scale=2.0**-12,
            )
            for h in (0, 1):
                t = t0 + h
                if causal and t // 4 == cq:
                    # diagonal block of tile t: valid iff sk<=sq
                    nc.vector.tensor_mul(
                        u3[:, t, ds(t * P, P)],
                        u3[:, t, ds(t * P, P)],
                        m01_sb[:, :],
                    )
                elif variant == "general":
                    mt = mt_pool.tile([P, CHUNK], BF16, tag="mt", name="mt")
                    nc.sync.dma_start(
                        mt[:, ds(0, w)], mT_d[ds(t * P, P), ds(a, w)]
                    )
                    nc.vector.tensor_mul(
                        u3[:, t, ds(a, w)],
                        u3[:, t, ds(a, w)],
                        mt[:, ds(0, w)],
                    )

        vch_tiles = [list(range(a // P, (a + L) // P)) for a, L in VCH]
        last_t0 = vch_tiles[-1][0]  # first sk-tile of the last v chunk
        vp_tiles = []
        out_ps = {}
        first_norm = [True]

        def emit_vp(t):
            pst = ps_proj.tile([P, CHUNK], F32, tag="ps_proj", name="ps_v")
            ps = pst[:, ds(0, DK)]
            for d in range(NDT):
                nc.tensor.matmul(
                    ps,
                    lhsT=vt3[:, d, ds(t * P, P)],
                    rhs=wv_sb[:, ts(d, DK)],
                    start=(d == 0),
                    stop=(d == NDT - 1),
                )
            vpt = vp_pool.tile([P, DK + 1], BF16, tag=f"vp{t}", name=f"vp{t}")
            nc.vector.tensor_copy(vpt[:, ds(0, DK)], ps)
            nc.vector.memset(vpt[:, ds(DK, 1)], 1.0)
            vp_tiles.append(vpt)

        def alloc_out(j):
            # one psum region per bank: a matmul accumulation start zeroes
            # beyond its own column region on HW, so banks can't be shared
            # between two in-flight output tiles.
            return (ps_out.tile([P, DK + 1], F32, tag="ps_out", name=f"ps_o{j}"), 0)

        def emit_out(j, tt_range, opst=None, stop_at=None):
            if opst is None:
                opst = alloc_out(j)
            tile_, base = opst
            stop_at = j if stop_at is None else stop_at
            for tt in tt_range:
                nc.tensor.matmul(
                    tile_[:, ds(base, DK + 1)],
                    lhsT=u3[:, tt, ds(j * P, P)],
                    rhs=vp_tiles[tt][:, :],
                    start=(tt == 0),
                    stop=(tt == stop_at),
                )
            return opst

        def emit_outj(j):
            if first_norm[0]:
                # bvb observation read before the first norm add
                nc.vector.tensor_copy(scr[:, ds(0, 1)], bvb_sb[:, ds(0, 1)])
                first_norm[0] = False
            opst = emit_out(j, range(j + 1))
            _norm_store(nc, osb_pool, opst, bvb_sb, out_d, j)

        for c in range(NCH):
            # emission order matches arrival order (k_c then q_c): pool slots
            # are granted in tile-creation order. A chunk's two diagonal
            # pairs are interleaved into the NEXT chunk's projection block,
            # so the ACT exp stream stays fed during the ~3.5us the PE spends
            # projecting. Before the c3 projections (which wait on kq3, the
            # last k/q data) the already-available v0 work is emitted.
            if causal and c == NCH - 1:
                with tc.tile_wait_until(G_V[0]):
                    sc_pair(2, 4)
                    emit_outj(2)
                    sc_pair(2, 5)
                    emit_outj(3)
            with tc.tile_wait_until(G_K[c]):
                proj_chunk(kt3, wk3, kpT, DK + 1, c, P, halves=2 if c == 0 else 1)
                if causal and c == 2:
                    sc_pair(c - 1, 2 * (c - 1))
            with tc.tile_wait_until(G_Q[c]):
                proj_chunk(qt3, wq3, qpT, DK, c, DK, halves=2 if c == 0 else 1)
                if c == 0 and causal:
                    # "Observation" read of m01 after the first proj copies so
                    # it doesn't block the DVE queue during startup, but
                    # before its first user (the c0 diagonal mask mults).
                    nc.vector.tensor_copy(scr[:, ds(1, 1)], m01_sb[:, ds(0, 1)])
                if causal:
                    if c == 2:
                        sc_pair(c - 1, 2 * (c - 1) + 1)
                    # chunk-c pairs: emit all for c=0, keep the two diagonal
                    # pairs for interleaving later (c3's pairs are all
                    # deferred into the v-chunk sequence). c2's inline pairs
                    # interleave with the vp0-3 projections (v0 is raised
                    # ahead of kq3 in the stream and lands just as the c2
                    # pairs start), filling the exp-lockstep stalls.
                    if c == 0:
                        for i in range(2):
                            sc_pair(c, i)
                    elif c == 1:
                        for i in range(2):
                            sc_pair(c, i)
                    elif c == 2:
                        for i in range(4):
                            sc_pair(c, i)
                            emit_vp(i)
                            if i >= 2:
                                emit_outj(i - 2)
                else:
                    # a pair (cq, i) needs qpT chunk cq AND kpT chunk i//2:
                    # emit it in group max(cq, i//2) so both already exist
                    pairs = [(c, i) for i in range(2 * c + 2)] + [
                        (cq, i)
                        for cq in range(c)
                        for i in (2 * c, 2 * c + 1)
                    ]
                    for cq, i in pairs:
                        sc_pair(cq, i)

        # --- deferred c3 scores + remaining v/out work, finely interleaved --
        # out[j] needs pieces (j//4, tt<=j) and vp tiles <= j. Alternating
        # the c3 score pairs (which rate-limit on the ACT exp drain) with vp
        # and out work keeps both engines fed.
        if causal:
            seq = (
                [("sc", 0), ("sc", 1),
                 ("vp", 4), ("sc", 2), ("vp", 5), ("sc", 3),
                 ("vp", 6), ("sc", 4), ("vp", 7),
                 ("out", 4), ("sc", 5), ("out", 5),
                 ("out", 6), ("sc", 6), ("out", 7), ("sc", 7),
                 ("vp", 8), ("vp", 9), ("vp", 10), ("vp", 11),
                 ("out", 8), ("out", 9), ("vp", 12), ("vp", 13),
                 ("out", 10), ("out", 11), ("out", 12), ("out", 13),
                 ("partial", 14), ("partial", 15),
                 ("vp", 14), ("vp", 15), ("final", 14), ("final", 15)]
            )
            ticks = {4: G_V[1], 8: G_V[2], 14: G_V[3]}
            tick = G_Q[-1] + 0.005
            for item in seq:
                kind, t = item
                if kind == "vp":
                    tick = max(tick, ticks.get(t, tick))
                with tc.tile_wait_until(tick):
                    if kind == "sc":
                        sc_pair(NCH - 1, t)
                    elif kind == "vp":
                        emit_vp(t)
                    elif kind == "out":
                        emit_outj(t)
                    elif kind == "partial":
                        # head start on the last chunk's outputs: accumulate
                        # the tt < last_t0 partials (u and vp already present)
                        out_ps[t] = emit_out(t, range(last_t0), stop_at=-1)
                    elif kind == "final":
                        opst = emit_out(t, range(last_t0, t + 1), opst=out_ps[t])
                        _norm_store(nc, osb_pool, opst, bvb_sb, out_d, t)
        else:
            for ci, tiles in enumerate(vch_tiles):
                with tc.tile_wait_until(G_V[ci]):
                    for t in tiles:
                        emit_vp(t)

        if not causal:
            with tc.tile_wait_until(G_V[-1]):
                for j in range(NST):
                    opst = emit_out(j, range(NST), stop_at=NST - 1)
                    _norm_store(nc, osb_pool, opst, bvb_sb, out_d, j)

    nc.compile()
    return nc


def _norm_store(nc, osb_pool, opst, bvb_sb, out_d, j):
    """normalize(out psum region) + bv -> DRAM (sync HWDGE ring, off the ACT
    queue so stores never serialize behind exp). opst = (psum tile, col base)."""
    tile_, base = opst
    rc = osb_pool.tile([P, 1], F32, tag=f"rc{j}", name=f"rc{j}")
    nc.vector.reciprocal(rc[:, :], tile_[:, ds(base + DK, 1)])
    osb = osb_pool.tile([P, DK], F32, tag=f"osb{j}", name=f"osb{j}")
    # out = psum * (1/den) + bv in one DVE op
    nc.vector.scalar_tensor_tensor(
        osb[:, :],
        tile_[:, ds(base, DK)],
        rc[:, :],
        bvb_sb[:, :],
        mybir.AluOpType.mult,
        mybir.AluOpType.add,
    )
    nc.sync.dma_start(out_d[ds(j * P, P), :], osb[:, :])


def _host_prep(Wq, bq, Wk, bk, Wv, bv):
    scale = np.float32(1.0 / np.sqrt(np.float32(DK)))
    Wq = np.asarray(Wq, np.float32)
    Wk = np.asarray(Wk, np.float32)
    Wv = np.asarray(Wv, np.float32)
    bq = np.asarray(bq, np.float32)
    bv = np.asarray(bv, np.float32)

    def relay(w, m, dt):
        return w.reshape(NDT, P, m).transpose(1, 0, 2).reshape(P, NDT * m).astype(dt)

    # q/k weights go to fp8 with a x64 gain: Wq*scale spans +-2^-8, below the
    # e4m3 normal range (min normal 2^-6); x64 shifts it to +-0.25. The gain
    # cancels in exp() via scale=2^-12 (64*64 on the qp.kp products) and the
    # qpT ones-row = 64 keeps the folded-bq column at the same scale.
    wq_r = relay(Wq * scale * 64.0, DK, F8NP)
    # bk is softmax-invariant (constant per query row) and dropped; bq folds
    # into an extra Wk column against the ones-row of qpT. Zero-padded to 128
    # columns for the DoubleRow stationary-width constraint.
    wk_aug = np.concatenate(
        [Wk, (Wk @ (bq * scale))[:, None], np.zeros((D, P - DK - 1), np.float32)],
        axis=1,
    )
    wk_r = relay(wk_aug * 64.0, P, F8NP)
    wv_r = relay(Wv, DK, BF)
    bvb = np.ascontiguousarray(np.broadcast_to(bv, (P, DK)))
    return wq_r, wk_r, wv_r, bvb


def _chunk_major(x, a, L, dt=BF):
    """[S, D] cols [a, a+L) -> [P, NDT, L]: arr[p,t,s] = x[a+s, 128t+p]."""
    return np.ascontiguousarray(
        np.asarray(x[a : a + L], np.float32)
        .astype(dt)
        .reshape(L, NDT, P)
        .transpose(2, 1, 0)
    )


_CACHE: dict = {}


def kernel(q, k, v, mask, Wq, bq, Wk, bk, Wv, bv):
    mask = np.asarray(mask)
    causal_ref = ~np.tril(np.ones((S, S), dtype=bool))
    if np.array_equal(mask, causal_ref):
        variant = "causal"
    elif not mask.any():
        variant = "full"
    else:
        variant = "general"

    wq_r, wk_r, wv_r, bvb = _host_prep(Wq, bq, Wk, bk, Wv, bv)
    m01 = np.triu(np.ones((P, P), np.float32)).astype(BF)

    in_maps = []
    for b in range(B):
        qb, kb, vb = np.asarray(q[b]), np.asarray(k[b]), np.asarray(v[b])
        hc = CHUNK // 2
        m = {
            "k0": np.ascontiguousarray(
                np.stack(
                    [_chunk_major(kb, 0, hc, F8NP), _chunk_major(kb, hc, hc, F8NP)],
                    axis=1,
                )
            ),
            "q0": np.ascontiguousarray(
                np.stack(
                    [_chunk_major(qb, 0, hc, F8NP), _chunk_major(qb, hc, hc, F8NP)],
                    axis=1,
                )
            ),
            "wq": wq_r,
            "wk": wk_r,
            "wv": wv_r,
            "bvb": bvb,
        }
        for c in range(1, NCH):
            m[f"kq{c}"] = np.ascontiguousarray(
                np.stack(
                    [
                        _chunk_major(kb, c * CHUNK, CHUNK, F8NP),
                        _chunk_major(qb, c * CHUNK, CHUNK, F8NP),
                    ],
                    axis=1,
                )
            )
        for i, (a, L) in enumerate(VCH):
            m[f"vT{i}"] = _chunk_major(vb, a, L)
        if variant == "causal":
            m["m01"] = m01
        if variant == "general":
            m["mT"] = np.ascontiguousarray((~mask).T.astype(BF))
        in_maps.append(m)

    if variant not in _CACHE:
        _CACHE[variant] = build(variant)
    nc = _CACHE[variant]

    res = run_bass_kernel_spmd(nc, in_maps, core_ids=list(range(NCORES)))
    out = np.stack([res.results[i]["out"] for i in range(NCORES)])
    return out.astype(np.float32)



# revision 69
# speedup vs baseline: 1.0428x; 1.0428x over previous
"""Trainium2 Bass kernel: single-head causal attention.

  out[b] = softmax(mask((q[b]Wq+bq)(k[b]Wk+bk)^T / sqrt(dk))) (v[b]Wv+bv)

Sharding: data-parallel over batch, one batch element per NeuronCore (B=8,
n_cores=8). No collectives. Host-side prep is layout + dtype staging +
parameter re-layout / algebraic folding:
  - q, k are pre-cast to fp8 e4m3 on host (2MB each per core vs 8MB f32);
    v stays bf16 (4MB) -- fp8 v alone pushes rel err past the 2e-2 gate.
  - Wq*scale and the bq-augmented Wk are fp8 with a x64 gain (their natural
    range is below the e4m3 normal minimum); the gain cancels inside exp()
    via scale=2^-12, with the qpT ones-row at 64 to keep the folded-bq
    column consistent. Wk is zero-padded to 128 columns (DoubleRow needs a
    32/64/128 stationary width; matmul time only depends on moving size).
  - bk drops out (softmax-invariant); bv is added after normalization.
  - End-to-end rel err 1.52e-2 (gate 2e-2), bit-matched by numsim3.py.

Per-core dataflow (S=2048, D=1024, dk=64, P=128):
  - inputs stream on the single-FIFO SWDGE ring in order k0 (2 half-DMAs:
    completion latency is per-DMA, so proj starts mid-arrival), q0 (2),
    kq1, kq2, v0, kq3, v1..v3. v0 is raised ahead of kq3: the PE only
    reaches the c3 projections ~3us after kq3 lands anyway, and early v0
    gives the PE vp/out work to fill exp-drain stalls.
  - k/q projections are fp8 DoubleRow (2 d-tiles = 256 contraction rows
    per matmul): half the PE cycles of bf16. 2-deep proj psum pool so the
    DVE psum-copy never stalls the next projection.
  - scores are emitted as PAIRS of k-tiles into [P, 2*CHUNK] 2-bank psum
    tiles; ONE ACTIVATE exps both pieces (3D AP into the single u_big
    [P, 16, S] tile) -- the ACT exp stream is the mid-kernel rate limiter
    and per-instruction overhead (~250ns) matters. The <=128 garbage
    below-diagonal columns a pair adds are never read downstream. Causal
    diagonal blocks masked by a 0/1 upper-tri multiply on DVE.
  - The kernel is PE-cycle-bound at the HAM-throttled ~1.2GHz clock
    (short 2.4GHz grants arrive sporadically), so emission order finely
    interleaves score pairs (exp-lockstepped) with vp projections and
    output tiles by expected data arrival, keeping PE and ACT both fed.
    A 6-matmul warm-up on a memset tile bridges preamble -> first data
    and opens the PE p-state ramp.
  - out[sq-tile j] = sum_t u[t]^T @ vp_t in PSUM [128,65]; vp column 64 is
    1 so the output matmul also emits the softmax denominator. j=14,15
    partial sums (t<=13) are accumulated one v-chunk early so only ~1us of
    compute trails the final DMA byte. Normalize with vector.reciprocal +
    tensor_scalar_mul, += bv, store on the sync HWDGE ring (off the ACT
    queue, which exp saturates).
  - tile_wait_until group tags keep Tile's static schedule aligned with
    the real FIFO arrival order (pool slots are granted in emission order,
    so emission must also match).
"""

import sys
from contextlib import ExitStack

import numpy as np

sys.path.insert(0, "/opt/trn_rl_repo")

import ml_dtypes  # noqa: E402

import concourse.mybir as mybir  # noqa: E402
import concourse.tile as tile  # noqa: E402
from concourse import bacc  # noqa: E402
from concourse.bass import ds, ts  # noqa: E402
from concourse.bass_utils import run_bass_kernel_spmd  # noqa: E402

S = 2048
D = 1024
DK = 64
P = 128
NDT = D // P  # 8 d-model tiles
NST = S // P  # 16 seq tiles
CHUNK = 512  # seq chunk = matmul moving-operand / PSUM-bank free size
NCH = S // CHUNK  # 4 column chunks for k/q
B = 8
NCORES = 8

# v chunk column spans (start, len): last chunk kept small so little work
# trails the final DMA byte
VCH = [(0, 512), (512, 512), (1024, 768), (1792, 256)]

F32 = mybir.dt.float32
BF16 = mybir.dt.bfloat16
F8 = mybir.dt.float8e4
BF = ml_dtypes.bfloat16
F8NP = ml_dtypes.float8_e4m3

# schedule group ticks (tile_wait_until "ms" units, used as logical order).
# k0 and q0 are separate loads (early exp start); k_c/q_c for c>=1 are
# merged interleaved pair-loads (fewer per-DMA completion stalls). v0 is
# streamed BEFORE kq3: the PE's queue only reaches the c3 projections ~3us
# after kq3 would land anyway, and an early v0 gives the PE vp/out work to
# fill the exp-drain stalls of the scores phase.
G_K = [0.01, 0.03, 0.04, 0.055]
G_Q = [0.02, 0.03, 0.04, 0.055]
G_V = [0.045, 0.07, 0.08, 0.09]


def build(variant: str = "causal") -> bacc.Bacc:
    """variant: 'causal' (skip sk>sq tiles, tri-mask diagonal),
    'full' (no masking), 'general' (arbitrary multiplicative mask)."""
    assert variant in ("causal", "full", "general")
    causal = variant == "causal"

    nc = bacc.Bacc()
    # k0/q0 are stored half-major ([P, 2, NDT, 256]) and loaded as two DMAs
    # each: SWDGE completion latency is per-DMA, so the first projection can
    # start ~halfway through the chunk's arrival.
    HC = CHUNK // 2
    k0_d = nc.declare_dram_parameter("k0", [P, 2, NDT, HC], F8, isOutput=False)
    q0_d = nc.declare_dram_parameter("q0", [P, 2, NDT, HC], F8, isOutput=False)
    kq_ds = [
        nc.declare_dram_parameter(f"kq{c}", [P, 2, NDT, CHUNK], F8, isOutput=False)
        for c in range(1, NCH)
    ]
    vT_ds = [
        nc.declare_dram_parameter(f"vT{i}", [P, NDT, L], BF16, isOutput=False)
        for i, (_, L) in enumerate(VCH)
    ]
    wq_d = nc.declare_dram_parameter("wq", [P, NDT * DK], F8, isOutput=False)
    # wk is padded to 128 columns (64 real + bq-fold + 63 zeros): DoubleRow
    # requires the stationary free size to be 32/64/128, and matmul time only
    # depends on the moving size, so the padding is free.
    wk_d = nc.declare_dram_parameter("wk", [P, NDT * P], F8, isOutput=False)
    wv_d = nc.declare_dram_parameter("wv", [P, NDT * DK], BF16, isOutput=False)
    bvb_d = nc.declare_dram_parameter("bvb", [P, DK], F32, isOutput=False)
    if causal:
        m01_d = nc.declare_dram_parameter("m01", [P, P], BF16, isOutput=False)
    if variant == "general":
        mT_d = nc.declare_dram_parameter("mT", [S, S], BF16, isOutput=False)
    out_d = nc.declare_dram_parameter("out", [S, DK], F32, isOutput=True)

    with ExitStack() as ctx:
        tc = ctx.enter_context(tile.TileContext(nc))
        const_pool = ctx.enter_context(tc.tile_pool(name="const", bufs=1))
        ld_pool = ctx.enter_context(tc.tile_pool(name="loads", bufs=1))
        pp_pool = ctx.enter_context(tc.tile_pool(name="projT", bufs=1))
        u_pool = ctx.enter_context(tc.tile_pool(name="u", bufs=1))
        vp_pool = ctx.enter_context(tc.tile_pool(name="vp", bufs=1))
        osb_pool = ctx.enter_context(tc.tile_pool(name="osb", bufs=1))
        scr_pool = ctx.enter_context(tc.tile_pool(name="scr", bufs=1))
        # proj pool has 2 bufs so consecutive projections (k/q alternation,
        # c0 halves) overlap their DVE psum-copy instead of stalling the PE
        # ~780ns each; the vp projections reuse this pool (projs are done by
        # the time v arrives).
        ps_proj = ctx.enter_context(tc.tile_pool(name="ps_proj", bufs=2, space="PSUM"))
        # scores psum: [P, 2*CHUNK] double tiles (2 banks) so one ACTIVATE
        # exps two score pieces — the ACT exp stream is the mid-kernel rate
        # limiter and its per-instruction overhead (~250ns) is substantial.
        ps_sc = ctx.enter_context(tc.tile_pool(name="ps_sc", bufs=2, space="PSUM"))
        ps_out = ctx.enter_context(tc.tile_pool(name="ps_out", bufs=2, space="PSUM"))

        # Warm-tile memset is the FIRST DVE instruction so the PE warm-up has
        # no dependency on any DMA (the observation reads of the consts would
        # otherwise block the DVE queue until the const ring delivers).
        warm = const_pool.tile([P, CHUNK], BF16, name="warm")
        nc.vector.memset(warm[:, :], 0.125)

        # --- constants (HWDGE sync ring), ordered by first use:
        # wk (proj k0 ~12us) -> wq (proj q0) -> m01 (first diag mask) ->
        # wv (vp, after the kq stream) -> bvb (norm, later still).
        wk_sb = const_pool.tile([P, NDT * P], F8, name="wk_sb")
        nc.sync.dma_start(wk_sb[:, :], wk_d[:, :])
        wq_sb = const_pool.tile([P, NDT * DK], F8, name="wq_sb")
        nc.sync.dma_start(wq_sb[:, :], wq_d[:, :])
        if causal:
            m01_sb = const_pool.tile([P, P], BF16, name="m01_sb")
            nc.sync.dma_start(m01_sb[:, :], m01_d[:, :])
        wv_sb = const_pool.tile([P, NDT * DK], BF16, name="wv_sb")
        nc.sync.dma_start(wv_sb[:, :], wv_d[:, :])
        bvb_sb = const_pool.tile([P, DK], F32, name="bvb_sb")
        nc.sync.dma_start(bvb_sb[:, :], bvb_d[:, :])

        scr = scr_pool.tile([P, 4], F32, name="scr")

        # --- big input loads ------------------------------------------------
        # SWDGE single FIFO ring with f32->bf16 cast in flight, strict order
        # k0, q0, kq1, kq2, kq3, v0..v3. k and q share one SBUF tile so the
        # merged pair chunks land in a single DMA.
        kqt = ld_pool.tile([P, 2 * NDT * S], F8, tag="kqt", name="kqt")
        vt = ld_pool.tile([P, NDT * S], BF16, tag="vt", name="vt")
        kq4 = kqt[:, :].rearrange("p (w t s) -> p w t s", w=2, s=S)
        kt3 = kq4[:, 0]
        qt3 = kq4[:, 1]
        vt3 = vt[:, :].rearrange("p (t s) -> p t s", s=S)

        with tc.tile_wait_until(G_K[0]):
            nc.gpsimd.dma_start(kt3[:, :, ds(0, HC)], k0_d[:, 0])
            nc.gpsimd.dma_start(kt3[:, :, ds(HC, HC)], k0_d[:, 1])
        with tc.tile_wait_until(G_Q[0]):
            nc.gpsimd.dma_start(qt3[:, :, ds(0, HC)], q0_d[:, 0])
            nc.gpsimd.dma_start(qt3[:, :, ds(HC, HC)], q0_d[:, 1])
        def load_kq(c):
            with tc.tile_wait_until(G_K[c]):
                nc.gpsimd.dma_start(
                    kq4[:, :, :, ds(c * CHUNK, CHUNK)], kq_ds[c - 1][:, :, :, :]
                )

        def load_v(i):
            a, L = VCH[i]
            with tc.tile_wait_until(G_V[i]):
                nc.gpsimd.dma_start(vt3[:, :, ds(a, L)], vT_ds[i][:, :, :])

        # ring FIFO order = emission order: kq1, kq2, v0, kq3, v1..v3
        load_kq(1)
        load_kq(2)
        load_v(0)
        load_kq(3)
        for i in range(1, len(VCH)):
            load_v(i)

        # PE warm-up: throwaway matmuls on a memset tile (no DMA dependency,
        # so they start right after the NEFF entry barrier) to open the PE
        # p-state ramp (0.65 -> 1.2 -> 2.4 GHz after ~3us of continuous
        # execution) before real work arrives, and to bridge the gap until
        # the first k chunk lands.
        with tc.tile_wait_until(0.005):
            wps = None
            for _ in range(6):
                wps = ps_sc.tile([P, 2 * CHUNK], F32, tag="ps_sc", name="ps_warm")
                nc.tensor.matmul(
                    wps[:, ds(0, CHUNK)],
                    lhsT=warm[:, ds(0, P)],
                    rhs=warm[:, ds(0, CHUNK)],
                    start=True,
                    stop=True,
                )
            nc.vector.tensor_copy(scr[:, ds(2, 1)], wps[:, ds(0, 1)])

        qpT = pp_pool.tile([DK + 1, S], BF16, tag="qpT", name="qpT")
        kpT = pp_pool.tile([DK + 1, S], BF16, tag="kpT", name="kpT")
        # weights carry a x64 gain (fp8 subnormal dodge); the ones-row of qpT
        # must match so the bq column lands at the same 64*64 scale as the
        # qp.kp products. exp() de-scales by 2^-12.
        nc.vector.memset(qpT[ds(DK, 1), :], 64.0)

        wq3 = wq_sb[:, :].rearrange("p (t m) -> p t m", m=DK)
        wk3 = wk_sb[:, :].rearrange("p (t m) -> p t m", m=P)

        def proj_chunk(src3, w3, dst, m, c, mm, halves=1):
            # fp8 DoubleRow: two d-tiles (256 contraction rows) per matmul.
            # mm = stationary free width (64 for q, 128 for padded k); m = rows
            # actually copied out. halves=2 projects the chunk in two column
            # halves (for c0, whose data lands as two DMAs).
            ps = ps_proj.tile([P, CHUNK], F32, tag="ps_proj", name="ps_p")
            L = CHUNK // halves
            for h in range(halves):
                for d in range(0, NDT, 2):
                    nc.tensor.matmul(
                        ps[:mm, ds(h * L, L)],
                        lhsT=w3[:, ds(d, 2), :],
                        rhs=src3[:, ds(d, 2), ds(c * CHUNK + h * L, L)],
                        start=(d == 0),
                        stop=(d == NDT - 2),
                        perf_mode=mybir.MatmulPerfMode.DoubleRow,
                    )
                nc.vector.tensor_copy(
                    dst[:m, ds(c * CHUNK + h * L, L)], ps[:m, ds(h * L, L)]
                )

        # --- scores + exp, sq-chunk-major so exp trails the q chunks -------
        if variant == "general":
            # mask slices are streamed per score pair (a resident [S, S] mask
            # no longer fits SBUF next to u_big); slow but correct.
            mt_pool = ctx.enter_context(tc.tile_pool(name="mt", bufs=2))

        # u is one big [P, NST, S] tile (columns = absolute sq) so a single
        # 3D-AP ACTIVATE can exp a PAIR of score pieces at once.
        u_big = u_pool.tile([P, NST * S], BF16, tag="u_big", name="u_big")
        u3 = u_big[:, :].rearrange("p (t s) -> p t s", s=S)

        def sc_pair(cq, i):
            """score matmuls + one shared exp for the piece pair
            (q-chunk cq, k-tiles 2i and 2i+1). Both pieces are computed from
            the same column start a (the earlier tile's causal start); the up
            to 128 below-diagonal columns this adds to tile 2i+1 are garbage
            that no out matmul ever reads."""
            t0 = 2 * i
            a = max(cq * CHUNK, t0 * P) if causal else cq * CHUNK
            w = (cq + 1) * CHUNK - a
            ps = ps_sc.tile([P, 2 * CHUNK], F32, tag="ps_sc", name="ps_s")
            for h in (0, 1):
                nc.tensor.matmul(
                    ps[:, ds(h * CHUNK, w)],
                    lhsT=kpT[:, ds((t0 + h) * P, P)],
                    rhs=qpT[:, ds(a, w)],
                    start=True,
                    stop=True,
                )
            ps3 = ps[:, :].rearrange("p (n s) -> p n s", n=2)
            nc.scalar.activation(
                u3[:, ds(t0, 2), ds(a, w)],
                ps3[:, :, ds(0, w)],
                mybir.ActivationFunctionType.Exp,
                scale=2.0**-12,
            )
            for h in (0, 1):
                t = t0 + h
                if causal and t // 4 == cq:
                    # diagonal block of tile t: valid iff sk<=sq
                    nc.vector.tensor_mul(
                        u3[:, t, ds(t * P, P)],
                        u3[:, t, ds(t * P, P)],
                        m01_sb[:, :],
                    )
                elif variant == "general":
                    mt = mt_pool.tile([P, CHUNK], BF16, tag="mt", name="mt")
                    nc.sync.dma_start(
                        mt[:, ds(0, w)], mT_d[ds(t * P, P), ds(a, w)]
                    )
                    nc.vector.tensor_mul(
                        u3[:, t, ds(a, w)],
                        u3[:, t, ds(a, w)],
                        mt[:, ds(0, w)],
                    )

        vch_tiles = [list(range(a // P, (a + L) // P)) for a, L in VCH]
        last_t0 = vch_tiles[-1][0]  # first sk-tile of the last v chunk
        vp_tiles = []
        out_ps = {}
        first_norm = [True]

        def emit_vp(t):
            pst = ps_proj.tile([P, CHUNK], F32, tag="ps_proj", name="ps_v")
            ps = pst[:, ds(0, DK)]
            for d in range(NDT):
                nc.tensor.matmul(
                    ps,
                    lhsT=vt3[:, d, ds(t * P, P)],
                    rhs=wv_sb[:, ts(d, DK)],
                    start=(d == 0),
                    stop=(d == NDT - 1),
                )
            vpt = vp_pool.tile([P, DK + 1], BF16, tag=f"vp{t}", name=f"vp{t}")
            nc.vector.tensor_copy(vpt[:, ds(0, DK)], ps)
            nc.vector.memset(vpt[:, ds(DK, 1)], 1.0)
            vp_tiles.append(vpt)

        def alloc_out(j):
            # one psum region per bank: a matmul accumulation start zeroes
            # beyond its own column region on HW, so banks can't be shared
            # between two in-flight output tiles.
            return (ps_out.tile([P, DK + 1], F32, tag="ps_out", name=f"ps_o{j}"), 0)

        def emit_out(j, tt_range, opst=None, stop_at=None):
            if opst is None:
                opst = alloc_out(j)
            tile_, base = opst
            stop_at = j if stop_at is None else stop_at
            for tt in tt_range:
                nc.tensor.matmul(
                    tile_[:, ds(base, DK + 1)],
                    lhsT=u3[:, tt, ds(j * P, P)],
                    rhs=vp_tiles[tt][:, :],
                    start=(tt == 0),
                    stop=(tt == stop_at),
                )
            return opst

        def emit_outj(j):
            if first_norm[0]:
                # bvb observation read before the first norm add
                nc.vector.tensor_copy(scr[:, ds(0, 1)], bvb_sb[:, ds(0, 1)])
                first_norm[0] = False
            opst = emit_out(j, range(j + 1))
            _norm_store(nc, osb_pool, opst, bvb_sb, out_d, j)

        for c in range(NCH):
            # emission order matches arrival order (k_c then q_c): pool slots
            # are granted in tile-creation order. A chunk's two diagonal
            # pairs are interleaved into the NEXT chunk's projection block,
            # so the ACT exp stream stays fed during the ~3.5us the PE spends
            # projecting. Before the c3 projections (which wait on kq3, the
            # last k/q data) the already-available v0 work is emitted.
            if causal and c == NCH - 1:
                with tc.tile_wait_until(G_V[0]):
                    sc_pair(2, 4)
                    emit_outj(2)
                    sc_pair(2, 5)
                    emit_outj(3)
            with tc.tile_wait_until(G_K[c]):
                proj_chunk(kt3, wk3, kpT, DK + 1, c, P, halves=2 if c == 0 else 1)
                if causal and c == 2:
                    sc_pair(c - 1, 2 * (c - 1))
            with tc.tile_wait_until(G_Q[c]):
                proj_chunk(qt3, wq3, qpT, DK, c, DK, halves=2 if c == 0 else 1)
                if c == 0 and causal:
                    # "Observation" read of m01 after the first proj copies so
                    # it doesn't block the DVE queue during startup, but
                    # before its first user (the c0 diagonal mask mults).
                    nc.vector.tensor_copy(scr[:, ds(1, 1)], m01_sb[:, ds(0, 1)])
                if causal:
                    if c == 2:
                        sc_pair(c - 1, 2 * (c - 1) + 1)
                    # chunk-c pairs: emit all for c=0, keep the two diagonal
                    # pairs for interleaving later (c3's pairs are all
                    # deferred into the v-chunk sequence). c2's inline pairs
                    # interleave with the vp0-3 projections (v0 is raised
                    # ahead of kq3 in the stream and lands just as the c2
                    # pairs start), filling the exp-lockstep stalls.
                    if c == 0:
                        for i in range(2):
                            sc_pair(c, i)
                    elif c == 1:
                        for i in range(2):
                            sc_pair(c, i)
                    elif c == 2:
                        for i in range(4):
                            sc_pair(c, i)
                            emit_vp(i)
                            if i >= 2:
                                emit_outj(i - 2)
                else:
                    # a pair (cq, i) needs qpT chunk cq AND kpT chunk i//2:
                    # emit it in group max(cq, i//2) so both already exist
                    pairs = [(c, i) for i in range(2 * c + 2)] + [
                        (cq, i)
                        for cq in range(c)
                        for i in (2 * c, 2 * c + 1)
                    ]
                    for cq, i in pairs:
                        sc_pair(cq, i)

        # --- deferred c3 scores + remaining v/out work, finely interleaved --
        # out[j] needs pieces (j//4, tt<=j) and vp tiles <= j. Alternating
        # the c3 score pairs (which rate-limit on the ACT exp drain) with vp
        # and out work keeps both engines fed.
        if causal:
            seq = (
                [("sc", 0), ("sc", 1),
                 ("vp", 4), ("sc", 2), ("vp", 5), ("sc", 3),
                 ("vp", 6), ("sc", 4), ("vp", 7),
                 ("out", 4), ("sc", 5), ("out", 5),
                 ("out", 6), ("sc", 6), ("out", 7),
                 ("vp", 8), ("vp", 9), ("vp", 10), ("vp", 11),
                 ("sc", 7),
                 ("out", 8), ("out", 9), ("vp", 12), ("vp", 13),
                 ("out", 10), ("out", 11), ("out", 12), ("out", 13),
                 ("partial", 14), ("partial", 15),
                 ("vp", 14), ("vp", 15), ("final", 14), ("final", 15)]
            )
            ticks = {4: G_V[1], 8: G_V[2], 14: G_V[3]}
            tick = G_Q[-1] + 0.005
            for item in seq:
                kind, t = item
                if kind == "vp":
                    tick = max(tick, ticks.get(t, tick))
                with tc.tile_wait_until(tick):
                    if kind == "sc":
                        sc_pair(NCH - 1, t)
                    elif kind == "vp":
                        emit_vp(t)
                    elif kind == "out":
                        emit_outj(t)
                    elif kind == "partial":
                        # head start on the last chunk's outputs: accumulate
                        # the tt < last_t0 partials (u and vp already present)
                        out_ps[t] = emit_out(t, range(last_t0), stop_at=-1)
                    elif kind == "final":
                        opst = emit_out(t, range(last_t0, t + 1), opst=out_ps[t])
                        _norm_store(nc, osb_pool, opst, bvb_sb, out_d, t)
        else:
            for ci, tiles in enumerate(vch_tiles):
                with tc.tile_wait_until(G_V[ci]):
                    for t in tiles:
                        emit_vp(t)

        if not causal:
            with tc.tile_wait_until(G_V[-1]):
                for j in range(NST):
                    opst = emit_out(j, range(NST), stop_at=NST - 1)
                    _norm_store(nc, osb_pool, opst, bvb_sb, out_d, j)

    nc.compile()
    return nc


def _norm_store(nc, osb_pool, opst, bvb_sb, out_d, j):
    """normalize(out psum region) + bv -> DRAM (sync HWDGE ring, off the ACT
    queue so stores never serialize behind exp). opst = (psum tile, col base)."""
    tile_, base = opst
    rc = osb_pool.tile([P, 1], F32, tag=f"rc{j}", name=f"rc{j}")
    nc.vector.reciprocal(rc[:, :], tile_[:, ds(base + DK, 1)])
    osb = osb_pool.tile([P, DK], F32, tag=f"osb{j}", name=f"osb{j}")
    # out = psum * (1/den) + bv in one DVE op
    nc.vector.scalar_tensor_tensor(
        osb[:, :],
        tile_[:, ds(base, DK)],
        rc[:, :],
        bvb_sb[:, :],
        mybir.AluOpType.mult,
        mybir.AluOpType.add,
    )
    nc.sync.dma_start(out_d[ds(j * P, P), :], osb[:, :])


def _host_prep(Wq, bq, Wk, bk, Wv, bv):
    scale = np.float32(1.0 / np.sqrt(np.float32(DK)))
    Wq = np.asarray(Wq, np.float32)
    Wk = np.asarray(Wk, np.float32)
    Wv = np.asarray(Wv, np.float32)
    bq = np.asarray(bq, np.float32)
    bv = np.asarray(bv, np.float32)

    def relay(w, m, dt):
        return w.reshape(NDT, P, m).transpose(1, 0, 2).reshape(P, NDT * m).astype(dt)

    # q/k weights go to fp8 with a x64 gain: Wq*scale spans +-2^-8, below the
    # e4m3 normal range (min normal 2^-6); x64 shifts it to +-0.25. The gain
    # cancels in exp() via scale=2^-12 (64*64 on the qp.kp products) and the
    # qpT ones-row = 64 keeps the folded-bq column at the same scale.
    wq_r = relay(Wq * scale * 64.0, DK, F8NP)
    # bk is softmax-invariant (constant per query row) and dropped; bq folds
    # into an extra Wk column against the ones-row of qpT. Zero-padded to 128
    # columns for the DoubleRow stationary-width constraint.
    wk_aug = np.concatenate(
        [Wk, (Wk @ (bq * scale))[:, None], np.zeros((D, P - DK - 1), np.float32)],
        axis=1,
    )
    wk_r = relay(wk_aug * 64.0, P, F8NP)
    wv_r = relay(Wv, DK, BF)
    bvb = np.ascontiguousarray(np.broadcast_to(bv, (P, DK)))
    return wq_r, wk_r, wv_r, bvb


def _chunk_major(x, a, L, dt=BF):
    """[S, D] cols [a, a+L) -> [P, NDT, L]: arr[p,t,s] = x[a+s, 128t+p]."""
    return np.ascontiguousarray(
        np.asarray(x[a : a + L], np.float32)
        .astype(dt)
        .reshape(L, NDT, P)
        .transpose(2, 1, 0)
    )


_CACHE: dict = {}


def kernel(q, k, v, mask, Wq, bq, Wk, bk, Wv, bv):
    mask = np.asarray(mask)
    causal_ref = ~np.tril(np.ones((S, S), dtype=bool))
    if np.array_equal(mask, causal_ref):
        variant = "causal"
    elif not mask.any():
        variant = "full"
    else:
        variant = "general"

    wq_r, wk_r, wv_r, bvb = _host_prep(Wq, bq, Wk, bk, Wv, bv)
    m01 = np.triu(np.ones((P, P), np.float32)).astype(BF)

    in_maps = []
    for b in range(B):
        qb, kb, vb = np.asarray(q[b]), np.asarray(k[b]), np.asarray(v[b])
        hc = CHUNK // 2
        m = {
            "k0": np.ascontiguousarray(
                np.stack(
                    [_chunk_major(kb, 0, hc, F8NP), _chunk_major(kb, hc, hc, F8NP)],
                    axis=1,
                )
            ),
            "q0": np.ascontiguousarray(
                np.stack(
                    [_chunk_major(qb, 0, hc, F8NP), _chunk_major(qb, hc, hc, F8NP)],
                    axis=1,
                )
            ),
            "wq": wq_r,
            "wk": wk_r,
            "wv": wv_r,
            "bvb": bvb,
        }
        for c in range(1, NCH):
            m[f"kq{c}"] = np.ascontiguousarray(
                np.stack(
                    [
                        _chunk_major(kb, c * CHUNK, CHUNK, F8NP),
                        _chunk_major(qb, c * CHUNK, CHUNK, F8NP),
                    ],
                    axis=1,
                )
            )
        for i, (a, L) in enumerate(VCH):
            m[f"vT{i}"] = _chunk_major(vb, a, L)
        if variant == "causal":
            m["m01"] = m01
        if variant == "general":
            m["mT"] = np.ascontiguousarray((~mask).T.astype(BF))
        in_maps.append(m)

    if variant not in _CACHE:
        _CACHE[variant] = build(variant)
    nc = _CACHE[variant]

    res = run_bass_kernel_spmd(nc, in_maps, core_ids=list(range(NCORES)))
    out = np.stack([res.results[i]["out"] for i in range(NCORES)])
    return out.astype(np.float32)

